# revision 86
# baseline (speedup 1.0000x reference)
"""LLaMA attention block (b=1, s=2048, d=2048, 16 heads) on 8 TRN2 NeuronCores.

Sharding: tensor-parallel over heads (2 heads per core). Each core computes
q/k/v projections for its head slice, RoPE, full (non-causal) attention for its
heads, and a partial output projection; the host sums the 8 partial outputs.

Device-side layout notes (per core):
  - x is passed transposed (xT, d-major) so projections contract over the
    partition dim without on-device transposes.
  - q/k are produced transposed per head: qT/kT [dh=128, s], RoPE'd in place
    (interleaved with the projection loop so PE never idles).
  - scores are computed transposed: scoresT [k, q]; exp (Act engine) evicts
    PSUM->SBUF as bf16 probs.
  - PV + row-sum are FUSED: rhs = [v | ones] (bf16 [128,129]); lhsT = probsT
    128-q chunk. Output osum is [q, dh+1] with the row-sum in column dh.
    This replaces the old M=1 ones-matmul (512 rows/block -> 1 row/block).
  - normalization is a per-partition scalar multiply at PSUM eviction
    (DVE tensor_scalar with rinv[128,1]); a cheap PE transpose (bf16,
    128 rows) restores the [dh, s] layout for the output projection.
  - probs/v/attn-out/wo are bf16 (rel err ~3e-3 vs gate 2e-2); projections
    and scores stay fp32r (full PE rate at N>=256).
  - output projection is interleaved into the attention loop (one [128,512]
    yp quarter per score group) so the PE stream has no phase boundary gaps
    (PE p-state halves the clock for 3us after any idle gap).
"""
import numpy as np
from contextlib import ExitStack

S, D, NH, DH = 2048, 2048, 16, 128
NCORES = 8
HPC = NH // NCORES          # heads per core
DHC = HPC * DH              # per-core projection width (256)
ROPE_BASE = 10000.0

_CACHE = {}
DEBUG = False


def _build(s, d):
    import concourse.bacc as bacc
    import concourse.mybir as mybir
    import concourse.tile as tile

    F32 = mybir.dt.float32
    F32R = mybir.dt.float32r
    BF16 = mybir.dt.bfloat16
    AF = mybir.ActivationFunctionType

    KB = d // 128          # contraction chunks for projections
    NS = s // 512          # s-slices for projections / q-slices for attention
    MB = s // 128          # s-blocks for output rows
    SCALE = 1.0 / float(np.sqrt(DH))

    def ksl(kb):
        return slice(128 * kb, 128 * (kb + 1))

    nc = bacc.Bacc("TRN2", target_bir_lowering=False, debug=False)

    # x and the packed qkv weights are partition-major so multi-chunk DMAs
    # iterate [p, chunk, ...] on both sides (fewer DMAs -> less per-DMA
    # fixed overhead), and bf16 (DMA transfers serialize globally in HW;
    # phase 1 is DMA-limited at fp32). bf16 projections still run 1 cyc/row.
    xT_d = nc.dram_tensor("xT2", [128, KB, s], BF16, kind="ExternalInput")
    wqkv_d = nc.dram_tensor("wqkvT", [128, KB, 3, DHC], BF16, kind="ExternalInput")
    wo_d = nc.dram_tensor("woT", [HPC, 128, s], BF16, kind="ExternalInput")
    cos_d = nc.dram_tensor("cosT", [128, s], F32, kind="ExternalInput")
    ssin_d = nc.dram_tensor("ssinT", [128, s], F32, kind="ExternalInput")
    perm_d = nc.dram_tensor("perm64", [128, 128], F32, kind="ExternalInput")
    ident_d = nc.dram_tensor("ident", [128, 128], BF16, kind="ExternalInput")
    # y partials in bf16: DMA transfers serialize globally, and the final
    # output-projection tail is DMA-bound at fp32 (host sums partials in f64)
    y_d = nc.dram_tensor("y", [MB, 128, s], BF16, kind="ExternalOutput")
    if DEBUG:
        dbg_rot = nc.dram_tensor("dbg_rot", [4, 128, s], F32, kind="ExternalOutput")
        dbg_v = nc.dram_tensor("dbg_v", [MB, 128, HPC, DH + 1], BF16,
                               kind="ExternalOutput")
        dbg_pr = nc.dram_tensor("dbg_pr", [128, 1024], BF16, kind="ExternalOutput")
        dbg_onat = nc.dram_tensor("dbg_onat", [128, 4, DH], BF16,
                                  kind="ExternalOutput")
        dbg_rinv = nc.dram_tensor("dbg_rinv", [2, 128, 2], F32,
                                  kind="ExternalOutput")
        dbg_oT = nc.dram_tensor("dbg_oT", [HPC, 128, s], BF16,
                                kind="ExternalOutput")

    with tile.TileContext(nc) as tc:
        with ExitStack() as root:
            consts = root.enter_context(tc.tile_pool(name="consts", bufs=1))
            perm_s = consts.tile([128, 128], F32R, name="perm_s")
            ident_s = consts.tile([128, 128], BF16, name="ident_s")
            cos_s = consts.tile([128, s], F32, name="cos_s")
            ssin_s = consts.tile([128, s], F32, name="ssin_s")
            wo_pool = root.enter_context(tc.tile_pool(name="wo_pool", bufs=1))
            wo_s = [wo_pool.tile([128, s], BF16, name=f"wo{h}") for h in range(HPC)]

            # v tiles hold [v_head0 | 1 | v_head1 | 1]: the ones column turns
            # the PV matmul into a fused PV+rowsum (out free dim 129).
            v_pool = root.enter_context(tc.tile_pool(name="v_pool", bufs=1))
            v_s = [v_pool.tile([128, HPC, DH + 1], BF16, name=f"v{i}")
                   for i in range(MB)]
            for i in range(MB):
                nc.vector.memset(v_s[i][:, :, DH:DH + 1], 1.0)

            rot_pool = root.enter_context(tc.tile_pool(name="rot_pool", bufs=1))
            qrot = [rot_pool.tile([128, s], F32R, name=f"qrot{m}") for m in range(HPC)]
            krot = [rot_pool.tile([128, s], F32R, name=f"krot{m}") for m in range(HPC)]

            oT_pool = root.enter_context(tc.tile_pool(name="oT_pool", bufs=1))
            oT_s = [oT_pool.tile([128, s], BF16, name=f"oT{h}") for h in range(HPC)]

            # qT/kT and the rope scratch pools live at root scope: the q1/k1
            # rope of the last two slices is deferred into early attention
            qkpre = root.enter_context(tc.tile_pool(name="qkpre", bufs=1))
            qT_s = [qkpre.tile([128, s], F32R, name=f"qT{m}") for m in range(HPC)]
            kT_s = [qkpre.tile([128, s], F32R, name=f"kT{m}") for m in range(HPC)]
            t1_pool = root.enter_context(tc.tile_pool(name="t1_pool", bufs=2))
            t2_pool = root.enter_context(tc.tile_pool(name="t2_pool", bufs=2))

            wqkv = root.enter_context(tc.tile_pool(name="wqkv", bufs=1))
            wqkv_s = wqkv.tile([128, KB, 3, DHC], BF16, name="wqkv_s")

            # ---------- phase 1: q/k/v projections with RoPE interleaved ----------
            ph1 = ExitStack()
            # kb 0 alone first so the PE can start as early as possible
            nc.scalar.dma_start(out=wqkv_s[:, 0:1, :, :], in_=wqkv_d[:, 0:1, :, :])
            nc.scalar.dma_start(out=wqkv_s[:, 1:2, :, :], in_=wqkv_d[:, 1:2, :, :])
            for p in range(1, KB // 2):
                nc.scalar.dma_start(out=wqkv_s[:, 2 * p:2 * p + 2, :, :],
                                    in_=wqkv_d[:, 2 * p:2 * p + 2, :, :])
            wq_s = [wqkv_s[:, i, 0, :] for i in range(KB)]
            wk_s = [wqkv_s[:, i, 1, :] for i in range(KB)]
            wv_s = [wqkv_s[:, i, 2, :] for i in range(KB)]

            xk_pool = ph1.enter_context(tc.tile_pool(name="xk_pool", bufs=3))
            qk_ps = ph1.enter_context(tc.tile_pool(name="qk_ps", bufs=1, space="PSUM"))
            v_ps = ph1.enter_context(tc.tile_pool(name="v_ps", bufs=1, space="PSUM"))
            rope_ps = ph1.enter_context(tc.tile_pool(name="rope_ps", bufs=2, space="PSUM"))

            # rope streams: k-head0 first so attention can start earliest
            streams = [(kT_s[0], krot[0]), (qT_s[0], qrot[0]),
                       (qT_s[1], qrot[1]), (kT_s[1], krot[1])]

            def emit_rope(nidx, only=(0, 1, 2, 3), ps=None, ps_tag="shift"):
                ns_ = slice(512 * nidx, 512 * (nidx + 1))
                for ri, (src, dst) in enumerate(streams):
                    if ri not in only:
                        continue
                    shift = (ps or rope_ps).tile([128, 512], F32,
                                                 name=f"sh{ri}_{nidx}", tag=ps_tag)
                    nc.tensor.matmul(shift[:], perm_s[:], src[:, ns_],
                                     start=True, stop=True)
                    t1 = t1_pool.tile([128, 512], F32, name=f"t1_{ri}_{nidx}", tag="t1")
                    nc.gpsimd.tensor_mul(t1[:], src[:, ns_].bitcast(F32), cos_s[:, ns_])
                    t2 = t2_pool.tile([128, 512], F32, name=f"t2_{ri}_{nidx}", tag="t2")
                    nc.vector.tensor_mul(t2[:], shift[:], ssin_s[:, ns_])
                    nc.vector.tensor_add(dst[:, ns_], t1[:], t2[:])

            for n in range(NS):
                ns_ = slice(512 * n, 512 * (n + 1))
                pq = [qk_ps.tile([128, 512], F32, name=f"pq{n}_{m}", tag=f"pq{m}")
                      for m in range(HPC)]
                pk = [qk_ps.tile([128, 512], F32, name=f"pk{n}_{m}", tag=f"pk{m}")
                      for m in range(HPC)]
                pv = [v_ps.tile([128, 2, 2, 128], F32, name=f"pv{n}_{t}", tag=f"pv{t}")
                      for t in range(2)]
                xkq = None
                for kb in range(KB):
                    if kb % 4 == 0:
                        xkq = xk_pool.tile([128, 4, 512], BF16,
                                           name=f"xk{n}_{kb // 4}", tag="xk")
                        if n == 0 and kb == 0:
                            # split so kb 0 lands quickly at startup
                            nc.sync.dma_start(out=xkq[:, 0:1, :],
                                              in_=xT_d[:, 0:1, ns_])
                            nc.sync.dma_start(out=xkq[:, 1:4, :],
                                              in_=xT_d[:, 1:4, ns_])
                        else:
                            nc.sync.dma_start(out=xkq[:],
                                              in_=xT_d[:, kb:kb + 4, ns_])
                    if kb == 8:
                        # cos/ssin arrive per-slice, interleaved between the
                        # x quads (slice n's columns are first read by rope(n)
                        # at the end of slice n+1's projection loop)
                        nc.scalar.dma_start(out=cos_s[:, ns_], in_=cos_d[:, ns_])
                        nc.sync.dma_start(out=ssin_s[:, ns_], in_=ssin_d[:, ns_])
                        if n == 0:
                            nc.scalar.dma_start(out=perm_s[:],
                                                in_=perm_d[:].bitcast(F32R))
                    xk = xkq[:, kb % 4, :]
                    st = kb == 0
                    sp = kb == KB - 1

                    def mm_q(m):
                        ms = slice(128 * m, 128 * (m + 1))
                        nc.tensor.matmul(pq[m][:], wq_s[kb][:, ms], xk,
                                         start=st, stop=sp)

                    def mm_k(m):
                        ms = slice(128 * m, 128 * (m + 1))
                        nc.tensor.matmul(pk[m][:], wk_s[kb][:, ms], xk,
                                         start=st, stop=sp)

                    def mm_v(j):
                        # psum start zeroes the whole bank: only the bank's
                        # first write (slot 0, kb 0) may set start=True
                        js = slice(128 * j, 128 * (j + 1))
                        nc.tensor.matmul(pv[j // 2][:, j % 2, :, :], xk[:, js],
                                         wv_s[kb][:], start=(st and j % 2 == 0),
                                         stop=sp)

                    if kb == 0 and n > 0:
                        # order matched to eviction completion of slice n-1
                        # (Act: pq0,pq1,pk1 / DVE: pk0 then pv0..3)
                        for f in (lambda: mm_q(0), lambda: mm_k(0),
                                  lambda: mm_q(1), lambda: mm_k(1),
                                  lambda: mm_v(0), lambda: mm_v(1),
                                  lambda: mm_v(2), lambda: mm_v(3)):
                            f()
                    else:
                        mm_q(0), mm_q(1), mm_k(0), mm_k(1)
                        for j in range(4):
                            mm_v(j)

                # evictions (only Act/DVE may read PSUM):
                # Act: pq0, pq1, pk1; DVE: pk0 first (rope k0 gate), then v.
                nc.vector.tensor_copy(kT_s[0][:, ns_], pk[0][:])
                nc.scalar.copy(qT_s[0][:, ns_], pq[0][:])
                nc.vector.tensor_copy(kT_s[1][:, ns_], pk[1][:])
                nc.scalar.copy(qT_s[1][:, ns_], pq[1][:])
                for j in range(4):
                    nc.vector.tensor_copy(v_s[4 * n + j][:, :, 0:DH],
                                          pv[j // 2][:, j % 2, :, :])
                if 0 < n < NS - 1:
                    emit_rope(n - 1)
            nc.sync.dma_start(out=ident_s[:], in_=ident_d[:])
            for h in range(HPC):
                nc.scalar.dma_start(out=wo_s[h][:], in_=wo_d[h])
            ph1.close()

            # -------- phase 3+4: attention with output projection interleaved ----
            ph3 = ExitStack()
            sc_ps = ph3.enter_context(tc.tile_pool(name="sc_ps", bufs=2, space="PSUM"))
            ops_ps = ph3.enter_context(tc.tile_pool(name="ops_ps", bufs=2, space="PSUM"))
            yp_ps = ph3.enter_context(tc.tile_pool(name="yp_ps", bufs=2, space="PSUM"))
            pr_pool = ph3.enter_context(tc.tile_pool(name="pr_pool", bufs=3))
            onat_pool = ph3.enter_context(tc.tile_pool(name="onat_pool", bufs=2))
            rinv_pool = ph3.enter_context(tc.tile_pool(name="rinv_pool", bufs=4))
            ysb_pool = ph3.enter_context(tc.tile_pool(name="ysb_pool", bufs=8))

            if DEBUG:
                for ri, (_, dst) in enumerate(streams):
                    nc.sync.dma_start(out=dbg_rot[ri], in_=dst[:].bitcast(F32))
                for i in range(MB):
                    nc.sync.dma_start(out=dbg_v[i], in_=v_s[i][:])

            # deferred v-projection of s-blocks 14/15: x columns re-fetched
            # (DMA pipe is idle at q0), accumulated in a yp bank (free now:
            # rope(2)/(3) is deferred too, so no shift reads pin these banks)
            xv_pool = ph3.enter_context(tc.tile_pool(name="xv_pool", bufs=1))
            xv_s = xv_pool.tile([128, KB, 2, 128], BF16, name="xv_s")
            nc.sync.dma_start(out=xv_s[:], in_=xT_d[:, :, 1792:2048])
            pv2 = yp_ps.tile([128, 2, 2, 128], F32, name="pv2", tag="yp")
            vunits = []
            for k in range(KB):
                def vunit(k=k):
                    for b in (0, 1):
                        nc.tensor.matmul(pv2[:, b, :, :], xv_s[:, k, b, :],
                                         wv_s[k][:],
                                         start=(k == 0 and b == 0),
                                         stop=(k == KB - 1))
                    if k == KB - 1:
                        nc.vector.tensor_copy(v_s[14][:, :, 0:DH],
                                              pv2[:, 0, :, :])
                        nc.vector.tensor_copy(v_s[15][:, :, 0:DH],
                                              pv2[:, 1, :, :])
                vunits.append(vunit)

            # rope(2)/(3) fillers pop during q0 (k-streams first: krot gates
            # attention); their shift tiles ride the sc pool's slots
            pending = [lambda nn=nn, ri=ri: emit_rope(nn, only=(ri,), ps=sc_ps,
                                                      ps_tag="sc")
                       for ri in (0, 3, 1, 2) for nn in (NS - 2, NS - 1)]
            finish_prev = [None]  # deferred transpose+evict of previous (h,q)

            def emit_quarter(mb, qt, eng):
                msl = slice(128 * mb, 128 * (mb + 1))
                cols = slice(512 * qt, 512 * (qt + 1))
                yp = yp_ps.tile([128, 512], F32, name=f"yp{mb}_{qt}", tag="yp")
                nc.tensor.matmul(yp[:], oT_s[0][:, msl], wo_s[0][:, cols],
                                 start=True, stop=False)
                nc.tensor.matmul(yp[:], oT_s[1][:, msl], wo_s[1][:, cols],
                                 start=False, stop=True)
                ys = ysb_pool.tile([128, 512], BF16, name=f"ys{mb}_{qt}", tag="ys")
                nc.vector.tensor_copy(ys[:], yp[:])
                if eng == 0:
                    nc.sync.dma_start(out=y_d[mb][:, cols], in_=ys[:])
                else:
                    nc.gpsimd.dma_start(out=y_d[mb][:, cols], in_=ys[:])

            for q in range(NS):
                qs = slice(512 * q, 512 * (q + 1))
                for h in range(HPC):
                    ot = [ops_ps.tile([128, 2, DH + 1], F32,
                                      name=f"osum{q}_{h}_{t}", tag="ops")
                          for t in range(2)]
                    pv_queue = []  # PV runs 2 groups behind sc to hide exp latency
                    for g in range(8):
                        sc_t = sc_ps.tile([128, 1024], F32,
                                          name=f"sc{q}_{h}_{g}", tag="sc")
                        nc.tensor.matmul(sc_t[:, 0:512], krot[h][:, ksl(2 * g)],
                                         qrot[h][:, qs], start=True, stop=True)
                        nc.tensor.matmul(sc_t[:, 512:1024],
                                         krot[h][:, ksl(2 * g + 1)],
                                         qrot[h][:, qs], start=True, stop=True)
                        pr_t = pr_pool.tile([128, 1024], BF16,
                                            name=f"pr{q}_{h}_{g}", tag="pr")
                        nc.scalar.activation(pr_t[:], sc_t[:], AF.Exp,
                                             scale=SCALE)
                        chunk = (lambda half, c, pr_t=pr_t:
                                 pr_t[:, 512 * half + 128 * c:
                                      512 * half + 128 * (c + 1)])
                        if DEBUG and q == 0 and h == 0 and g == 0:
                            nc.sync.dma_start(out=dbg_pr[:], in_=pr_t[:])
                        if g == 3 and finish_prev[0] is not None:
                            finish_prev[0]()
                            finish_prev[0] = None
                        # interleave fillers: at q0, rope(2)/(3) units every
                        # group plus 2 deferred-v units per h0 group; at q1+,
                        # yp quarters (skip g7 to keep DVE clear for the
                        # h-boundary normalize chain)
                        def may_pop():
                            return pending and not (q == NS - 1
                                                    and len(pending) <= 3)
                        if q == 0:
                            if pending:
                                pending.pop(0)()
                            if h == 0 and vunits:
                                vunits.pop(0)()
                                if vunits:
                                    vunits.pop(0)()
                        elif not (h == 0 and g <= 3) and g != 7:
                            if may_pop():
                                pending.pop(0)()
                            if g in (2, 4, 5) and may_pop():
                                pending.pop(0)()

                        def pv_emit(g=g, chunk=chunk):
                            for half in range(2):
                                kb = 2 * g + half
                                for c in range(4):
                                    nc.tensor.matmul(
                                        ot[c // 2][:, c % 2, :],
                                        chunk(half, c), v_s[kb][:, h, :],
                                        start=(kb == 0 and c % 2 == 0),
                                        stop=(kb == KB - 1))
                        pv_queue.append(pv_emit)
                        if len(pv_queue) > 2:
                            pv_queue.pop(0)()
                    for f in pv_queue:
                        f()

                    # normalization: rinv from the fused row-sum column, applied
                    # per-partition while evicting to bf16
                    rinv_t = [rinv_pool.tile([128, 2, 1], F32,
                                             name=f"rinv{q}_{h}_{t}", tag="rinv")
                              for t in range(2)]
                    for t in range(2):
                        nc.vector.reciprocal_approx_fast(rinv_t[t][:, :, :],
                                                         ot[t][:, :, DH:DH + 1])
                    onat = onat_pool.tile([128, 4, DH], BF16, name=f"onat{q}_{h}",
                                          tag="onat")
                    for c in range(4):
                        nc.vector.tensor_scalar_mul(onat[:, c, :],
                                                    ot[c // 2][:, c % 2, 0:DH],
                                                    rinv_t[c // 2][:, c % 2, :])
                    if DEBUG and q == 0 and h == 0:
                        for t in range(2):
                            nc.sync.dma_start(out=dbg_rinv[t],
                                              in_=rinv_t[t][:, :, 0])
                        nc.sync.dma_start(out=dbg_onat[:], in_=onat[:])
                    tr = sc_ps.tile([128, 4, DH], BF16, name=f"tr{q}_{h}", tag="sc")

                    def finish(q=q, h=h, onat=onat, tr=tr):
                        for c in range(4):
                            nc.tensor.matmul(tr[:, c, :], onat[:, c, :],
                                             ident_s[:], is_transpose=True,
                                             start=(c == 0), stop=(c == 3))
                        for c in range(4):
                            cols = slice(512 * q + 128 * c, 512 * q + 128 * (c + 1))
                            nc.vector.tensor_copy(oT_s[h][:, cols], tr[:, c, :])
                    finish_prev[0] = finish

                # queue this q-slice's output-projection quarters (the last
                # q-slice is instead emitted as double-width halves in the tail)
                if q < NS - 1:
                    eng = 0
                    for mb in range(4 * q, 4 * q + 4):
                        for qt in range(4):
                            pending.append(lambda mb=mb, qt=qt, e=eng:
                                           emit_quarter(mb, qt, e))
                            eng ^= 1

            # tail: the 3 reserved q2 quarters cover the PE gap while the last
            # (h,q)'s normalize chain completes, then its transposes run, then
            # the last q-slice's output projection streams as [128,1024] halves
            # through the freed sc-pool slots (double-buffered, no evict stall).
            for p in pending:
                p()
            pending = []
            finish_prev[0]()
            finish_prev[0] = None
            if DEBUG:
                for hh in range(HPC):
                    nc.sync.dma_start(out=dbg_oT[hh], in_=oT_s[hh][:])
            for mb in range(4 * (NS - 1), 4 * NS):
                for hf in range(2):
                    yph = sc_ps.tile([128, 1024], F32, name=f"yph{mb}_{hf}", tag="sc")
                    for h in range(HPC):
                        for nn in range(2):
                            cols = slice(1024 * hf + 512 * nn,
                                         1024 * hf + 512 * (nn + 1))
                            nc.tensor.matmul(yph[:, 512 * nn:512 * (nn + 1)],
                                             oT_s[h][:, slice(128 * mb, 128 * (mb + 1))],
                                             wo_s[h][:, cols],
                                             start=(h == 0), stop=(h == HPC - 1))
                    ysh = ysb_pool.tile([128, 1024], BF16, name=f"ysh{mb}_{hf}",
                                        tag="ysh")
                    if (2 * mb + hf) % 2 == 0:
                        nc.scalar.copy(ysh[:], yph[:])
                    else:
                        nc.vector.tensor_copy(ysh[:], yph[:])
                    nc.sync.dma_start(out=y_d[mb][:, 1024 * hf:1024 * hf + 512],
                                      in_=ysh[:, 0:512])
                    nc.gpsimd.dma_start(out=y_d[mb][:, 1024 * hf + 512:1024 * (hf + 1)],
                                        in_=ysh[:, 512:1024])
            ph3.close()

    nc.compile()
    return nc


def _prepare_inputs(hidden_states, wq, wk, wv, wo, position_ids, s, d):
    """Host-side sharding/layout prep. Returns per-core input maps."""
    import ml_dtypes

    x = np.asarray(hidden_states, np.float32).reshape(s, d)
    kb = d // 128
    # partition-major bf16: [128 rows-within-chunk, kb, s]
    xT = np.ascontiguousarray(
        x.T.reshape(kb, 128, s).transpose(1, 0, 2)).astype(ml_dtypes.bfloat16)

    pos = np.asarray(position_ids).reshape(-1)[:s].astype(np.float64)
    inv_freq = 1.0 / (ROPE_BASE ** (np.arange(0, DH, 2, dtype=np.float64) / DH))
    freqs = np.outer(pos, inv_freq)                      # [s, dh/2]
    emb = np.concatenate([freqs, freqs], axis=-1)        # [s, dh]
    cosT = np.ascontiguousarray(np.cos(emb).T.astype(np.float32))   # [dh, s]
    sinT = np.ascontiguousarray(np.sin(emb).T.astype(np.float32))
    ssinT = sinT.copy()
    ssinT[: DH // 2] *= -1.0

    perm64 = np.zeros((128, 128), np.float32)
    for m in range(128):
        perm64[(m + 64) % 128, m] = 1.0
    ident = np.eye(128, dtype=ml_dtypes.bfloat16)

    wq = np.asarray(wq, np.float32)
    wk = np.asarray(wk, np.float32)
    wv = np.asarray(wv, np.float32)
    wo = np.asarray(wo, np.float32)

    in_maps = []
    for c in range(NCORES):
        cs = slice(DHC * c, DHC * (c + 1))
        wqT = wq[cs, :].T.reshape(kb, 128, DHC)
        wkT = wk[cs, :].T.reshape(kb, 128, DHC)
        wvT = wv[cs, :].T.reshape(kb, 128, DHC)
        # packed bf16 [128, kb, 3, DHC]
        wqkvT = np.ascontiguousarray(
            np.stack([wqT, wkT, wvT], axis=1).transpose(2, 0, 1, 3)
        ).astype(ml_dtypes.bfloat16)
        woT = np.ascontiguousarray(wo[:, cs].T).reshape(HPC, 128, d)
        woT = woT.astype(ml_dtypes.bfloat16)
        in_maps.append(dict(
            xT2=xT, wqkvT=wqkvT, woT=woT,
            cosT=cosT, ssinT=ssinT,
            perm64=perm64, ident=ident,
        ))
    return in_maps


def kernel(hidden_states, wq, wk, wv, wo, position_ids):
    from concourse.bass_utils import run_bass_kernel_spmd

    b, s, d = hidden_states.shape
    if "nc" not in _CACHE:
        _CACHE["nc"] = _build(s, d)
    nc = _CACHE["nc"]

    in_maps = _prepare_inputs(hidden_states, wq, wk, wv, wo, position_ids, s, d)
    res = None
    last_err = None
    for attempt in range(3):
        try:
            res = run_bass_kernel_spmd(nc, in_maps, core_ids=list(range(NCORES)))
            break
        except Exception as e:  # transient device/terminal failures happen
            last_err = e
            import time as _time
            _time.sleep(5.0)
    if res is None:
        raise last_err
    y = np.zeros((s, d), np.float64)
    for c in range(NCORES):
        y += res.results[c]["y"].reshape(s, d).astype(np.float64)
    return y.astype(np.float32).reshape(b, s, d)


# revision 87
# speedup vs baseline: 1.0044x; 1.0044x over previous
"""LLaMA attention block (b=1, s=2048, d=2048, 16 heads) on 8 TRN2 NeuronCores.

Sharding: tensor-parallel over heads (2 heads per core). Each core computes
q/k/v projections for its head slice, RoPE, full (non-causal) attention for its
heads, and a partial output projection; the host sums the 8 partial outputs.

Device-side layout notes (per core):
  - x is passed transposed (xT, d-major) so projections contract over the
    partition dim without on-device transposes.
  - q/k are produced transposed per head: qT/kT [dh=128, s], RoPE'd in place
    (interleaved with the projection loop so PE never idles).
  - scores are computed transposed: scoresT [k, q]; exp (Act engine) evicts
    PSUM->SBUF as bf16 probs.
  - PV + row-sum are FUSED: rhs = [v | ones] (bf16 [128,129]); lhsT = probsT
    128-q chunk. Output osum is [q, dh+1] with the row-sum in column dh.
    This replaces the old M=1 ones-matmul (512 rows/block -> 1 row/block).
  - normalization is a per-partition scalar multiply at PSUM eviction
    (DVE tensor_scalar with rinv[128,1]); a cheap PE transpose (bf16,
    128 rows) restores the [dh, s] layout for the output projection.
  - probs/v/attn-out/wo are bf16 (rel err ~3e-3 vs gate 2e-2); projections
    and scores stay fp32r (full PE rate at N>=256).
  - output projection is interleaved into the attention loop (one [128,512]
    yp quarter per score group) so the PE stream has no phase boundary gaps
    (PE p-state halves the clock for 3us after any idle gap).
"""
import numpy as np
from contextlib import ExitStack

S, D, NH, DH = 2048, 2048, 16, 128
NCORES = 8
HPC = NH // NCORES          # heads per core
DHC = HPC * DH              # per-core projection width (256)
ROPE_BASE = 10000.0

_CACHE = {}
DEBUG = False


def _build(s, d):
    import concourse.bacc as bacc
    import concourse.mybir as mybir
    import concourse.tile as tile

    F32 = mybir.dt.float32
    F32R = mybir.dt.float32r
    BF16 = mybir.dt.bfloat16
    AF = mybir.ActivationFunctionType

    KB = d // 128          # contraction chunks for projections
    NS = s // 512          # s-slices for projections / q-slices for attention
    MB = s // 128          # s-blocks for output rows
    SCALE = 1.0 / float(np.sqrt(DH))

    def ksl(kb):
        return slice(128 * kb, 128 * (kb + 1))

    nc = bacc.Bacc("TRN2", target_bir_lowering=False, debug=False)

    # x and the packed qkv weights are partition-major so multi-chunk DMAs
    # iterate [p, chunk, ...] on both sides (fewer DMAs -> less per-DMA
    # fixed overhead), and bf16 (DMA transfers serialize globally in HW;
    # phase 1 is DMA-limited at fp32). bf16 projections still run 1 cyc/row.
    xT_d = nc.dram_tensor("xT2", [128, KB, s], BF16, kind="ExternalInput")
    wqkv_d = nc.dram_tensor("wqkvT", [128, KB, 3, DHC], BF16, kind="ExternalInput")
    wo_d = nc.dram_tensor("woT", [HPC, 128, s], BF16, kind="ExternalInput")
    cos_d = nc.dram_tensor("cosT", [128, s], F32, kind="ExternalInput")
    ssin_d = nc.dram_tensor("ssinT", [128, s], F32, kind="ExternalInput")
    perm_d = nc.dram_tensor("perm64", [128, 128], F32, kind="ExternalInput")
    ident_d = nc.dram_tensor("ident", [128, 128], BF16, kind="ExternalInput")
    # y partials in bf16: DMA transfers serialize globally, and the final
    # output-projection tail is DMA-bound at fp32 (host sums partials in f64)
    y_d = nc.dram_tensor("y", [MB, 128, s], BF16, kind="ExternalOutput")
    if DEBUG:
        dbg_rot = nc.dram_tensor("dbg_rot", [4, 128, s], F32, kind="ExternalOutput")
        dbg_v = nc.dram_tensor("dbg_v", [MB, 128, HPC, DH + 1], BF16,
                               kind="ExternalOutput")
        dbg_pr = nc.dram_tensor("dbg_pr", [128, 1024], BF16, kind="ExternalOutput")
        dbg_onat = nc.dram_tensor("dbg_onat", [128, 4, DH], BF16,
                                  kind="ExternalOutput")
        dbg_rinv = nc.dram_tensor("dbg_rinv", [2, 128, 2], F32,
                                  kind="ExternalOutput")
        dbg_oT = nc.dram_tensor("dbg_oT", [HPC, 128, s], BF16,
                                kind="ExternalOutput")

    with tile.TileContext(nc) as tc:
        with ExitStack() as root:
            consts = root.enter_context(tc.tile_pool(name="consts", bufs=1))
            perm_s = consts.tile([128, 128], F32R, name="perm_s")
            ident_s = consts.tile([128, 128], BF16, name="ident_s")
            cos_s = consts.tile([128, s], F32, name="cos_s")
            ssin_s = consts.tile([128, s], F32, name="ssin_s")
            wo_pool = root.enter_context(tc.tile_pool(name="wo_pool", bufs=1))
            wo_s = [wo_pool.tile([128, s], BF16, name=f"wo{h}") for h in range(HPC)]

            # v tiles hold [v_head0 | 1 | v_head1 | 1]: the ones column turns
            # the PV matmul into a fused PV+rowsum (out free dim 129).
            v_pool = root.enter_context(tc.tile_pool(name="v_pool", bufs=1))
            v_s = [v_pool.tile([128, HPC, DH + 1], BF16, name=f"v{i}")
                   for i in range(MB)]
            for i in range(MB):
                nc.vector.memset(v_s[i][:, :, DH:DH + 1], 1.0)

            rot_pool = root.enter_context(tc.tile_pool(name="rot_pool", bufs=1))
            qrot = [rot_pool.tile([128, s], F32R, name=f"qrot{m}") for m in range(HPC)]
            krot = [rot_pool.tile([128, s], F32R, name=f"krot{m}") for m in range(HPC)]

            oT_pool = root.enter_context(tc.tile_pool(name="oT_pool", bufs=1))
            oT_s = [oT_pool.tile([128, s], BF16, name=f"oT{h}") for h in range(HPC)]

            # qT/kT and the rope scratch pools live at root scope: the q1/k1
            # rope of the last two slices is deferred into early attention
            qkpre = root.enter_context(tc.tile_pool(name="qkpre", bufs=1))
            qT_s = [qkpre.tile([128, s], F32R, name=f"qT{m}") for m in range(HPC)]
            kT_s = [qkpre.tile([128, s], F32R, name=f"kT{m}") for m in range(HPC)]
            t1_pool = root.enter_context(tc.tile_pool(name="t1_pool", bufs=2))
            t2_pool = root.enter_context(tc.tile_pool(name="t2_pool", bufs=2))
            xv_pool = root.enter_context(tc.tile_pool(name="xv_pool", bufs=1))
            xv_s = xv_pool.tile([128, KB, 2, 128], BF16, name="xv_s")

            wqkv = root.enter_context(tc.tile_pool(name="wqkv", bufs=1))
            wqkv_s = wqkv.tile([128, KB, 3, DHC], BF16, name="wqkv_s")

            # ---------- phase 1: q/k/v projections with RoPE interleaved ----------
            ph1 = ExitStack()
            # kb 0 alone first so the PE can start as early as possible
            nc.scalar.dma_start(out=wqkv_s[:, 0:1, :, :], in_=wqkv_d[:, 0:1, :, :])
            nc.scalar.dma_start(out=wqkv_s[:, 1:2, :, :], in_=wqkv_d[:, 1:2, :, :])
            for p in range(1, KB // 2):
                nc.scalar.dma_start(out=wqkv_s[:, 2 * p:2 * p + 2, :, :],
                                    in_=wqkv_d[:, 2 * p:2 * p + 2, :, :])
            wq_s = [wqkv_s[:, i, 0, :] for i in range(KB)]
            wk_s = [wqkv_s[:, i, 1, :] for i in range(KB)]
            wv_s = [wqkv_s[:, i, 2, :] for i in range(KB)]

            xk_pool = ph1.enter_context(tc.tile_pool(name="xk_pool", bufs=3))
            qk_ps = ph1.enter_context(tc.tile_pool(name="qk_ps", bufs=1, space="PSUM"))
            v_ps = ph1.enter_context(tc.tile_pool(name="v_ps", bufs=1, space="PSUM"))
            rope_ps = ph1.enter_context(tc.tile_pool(name="rope_ps", bufs=2, space="PSUM"))

            # rope streams: k-head0 first so attention can start earliest
            streams = [(kT_s[0], krot[0]), (qT_s[0], qrot[0]),
                       (qT_s[1], qrot[1]), (kT_s[1], krot[1])]

            def emit_rope(nidx, only=(0, 1, 2, 3), ps=None, ps_tag="shift"):
                ns_ = slice(512 * nidx, 512 * (nidx + 1))
                for ri, (src, dst) in enumerate(streams):
                    if ri not in only:
                        continue
                    shift = (ps or rope_ps).tile([128, 512], F32,
                                                 name=f"sh{ri}_{nidx}", tag=ps_tag)
                    nc.tensor.matmul(shift[:], perm_s[:], src[:, ns_],
                                     start=True, stop=True)
                    t1 = t1_pool.tile([128, 512], F32, name=f"t1_{ri}_{nidx}", tag="t1")
                    nc.gpsimd.tensor_mul(t1[:], src[:, ns_].bitcast(F32), cos_s[:, ns_])
                    t2 = t2_pool.tile([128, 512], F32, name=f"t2_{ri}_{nidx}", tag="t2")
                    nc.vector.tensor_mul(t2[:], shift[:], ssin_s[:, ns_])
                    nc.vector.tensor_add(dst[:, ns_], t1[:], t2[:])

            for n in range(NS):
                ns_ = slice(512 * n, 512 * (n + 1))
                pq = [qk_ps.tile([128, 512], F32, name=f"pq{n}_{m}", tag=f"pq{m}")
                      for m in range(HPC)]
                pk = [qk_ps.tile([128, 512], F32, name=f"pk{n}_{m}", tag=f"pk{m}")
                      for m in range(HPC)]
                pv = [v_ps.tile([128, 2, 2, 128], F32, name=f"pv{n}_{t}", tag=f"pv{t}")
                      for t in range(2)]
                xkq = None
                for kb in range(KB):
                    if kb % 4 == 0:
                        xkq = xk_pool.tile([128, 4, 512], BF16,
                                           name=f"xk{n}_{kb // 4}", tag="xk")
                        if n == 0 and kb == 0:
                            # split so kb 0 lands quickly at startup
                            nc.sync.dma_start(out=xkq[:, 0:1, :],
                                              in_=xT_d[:, 0:1, ns_])
                            nc.sync.dma_start(out=xkq[:, 1:4, :],
                                              in_=xT_d[:, 1:4, ns_])
                        else:
                            nc.sync.dma_start(out=xkq[:],
                                              in_=xT_d[:, kb:kb + 4, ns_])
                    if kb == 8:
                        # cos/ssin arrive per-slice, interleaved between the
                        # x quads (slice n's columns are first read by rope(n)
                        # at the end of slice n+1's projection loop)
                        nc.scalar.dma_start(out=cos_s[:, ns_], in_=cos_d[:, ns_])
                        nc.sync.dma_start(out=ssin_s[:, ns_], in_=ssin_d[:, ns_])
                        if n == 0:
                            nc.scalar.dma_start(out=perm_s[:],
                                                in_=perm_d[:].bitcast(F32R))
                    xk = xkq[:, kb % 4, :]
                    st = kb == 0
                    sp = kb == KB - 1

                    def mm_q(m):
                        ms = slice(128 * m, 128 * (m + 1))
                        nc.tensor.matmul(pq[m][:], wq_s[kb][:, ms], xk,
                                         start=st, stop=sp)

                    def mm_k(m):
                        ms = slice(128 * m, 128 * (m + 1))
                        nc.tensor.matmul(pk[m][:], wk_s[kb][:, ms], xk,
                                         start=st, stop=sp)

                    def mm_v(j):
                        # psum start zeroes the whole bank: only the bank's
                        # first write (slot 0, kb 0) may set start=True
                        js = slice(128 * j, 128 * (j + 1))
                        nc.tensor.matmul(pv[j // 2][:, j % 2, :, :], xk[:, js],
                                         wv_s[kb][:], start=(st and j % 2 == 0),
                                         stop=sp)

                    if kb == 0 and n > 0:
                        # order matched to eviction completion of slice n-1
                        # (Act: pq0,pq1,pk1 / DVE: pk0 then pv0..3)
                        for f in (lambda: mm_q(0), lambda: mm_k(0),
                                  lambda: mm_q(1), lambda: mm_k(1),
                                  lambda: mm_v(0), lambda: mm_v(1),
                                  lambda: mm_v(2), lambda: mm_v(3)):
                            f()
                    else:
                        mm_q(0), mm_q(1), mm_k(0), mm_k(1)
                        for j in range(4):
                            mm_v(j)

                # evictions (only Act/DVE may read PSUM):
                # Act: pq0, pq1, pk1; DVE: pk0 first (rope k0 gate), then v.
                nc.vector.tensor_copy(kT_s[0][:, ns_], pk[0][:])
                nc.scalar.copy(qT_s[0][:, ns_], pq[0][:])
                nc.vector.tensor_copy(kT_s[1][:, ns_], pk[1][:])
                nc.scalar.copy(qT_s[1][:, ns_], pq[1][:])
                for j in range(4):
                    nc.vector.tensor_copy(v_s[4 * n + j][:, :, 0:DH],
                                          pv[j // 2][:, j % 2, :, :])
                if n == NS - 2:
                    # prefetch x columns for the q0-deferred v blocks 14/15
                    nc.gpsimd.dma_start(out=xv_s[:], in_=xT_d[:, :, 1792:2048])
                if 0 < n < NS - 1:
                    emit_rope(n - 1)
            nc.sync.dma_start(out=ident_s[:], in_=ident_d[:])
            for h in range(HPC):
                nc.scalar.dma_start(out=wo_s[h][:], in_=wo_d[h])
            ph1.close()

            # -------- phase 3+4: attention with output projection interleaved ----
            ph3 = ExitStack()
            sc_ps = ph3.enter_context(tc.tile_pool(name="sc_ps", bufs=2, space="PSUM"))
            ops_ps = ph3.enter_context(tc.tile_pool(name="ops_ps", bufs=2, space="PSUM"))
            yp_ps = ph3.enter_context(tc.tile_pool(name="yp_ps", bufs=2, space="PSUM"))
            pr_pool = ph3.enter_context(tc.tile_pool(name="pr_pool", bufs=3))
            onat_pool = ph3.enter_context(tc.tile_pool(name="onat_pool", bufs=2))
            rinv_pool = ph3.enter_context(tc.tile_pool(name="rinv_pool", bufs=4))
            ysb_pool = ph3.enter_context(tc.tile_pool(name="ysb_pool", bufs=8))

            if DEBUG:
                for ri, (_, dst) in enumerate(streams):
                    nc.sync.dma_start(out=dbg_rot[ri], in_=dst[:].bitcast(F32))
                for i in range(MB):
                    nc.sync.dma_start(out=dbg_v[i], in_=v_s[i][:])

            # deferred v-projection of s-blocks 14/15: x columns re-fetched
            # (DMA pipe is idle at q0), accumulated in a yp bank (free now:
            # rope(2)/(3) is deferred too, so no shift reads pin these banks)
            pv2 = yp_ps.tile([128, 2, 2, 128], F32, name="pv2", tag="yp")
            vunits = []
            for k in range(KB):
                def vunit(k=k):
                    for b in (0, 1):
                        nc.tensor.matmul(pv2[:, b, :, :], xv_s[:, k, b, :],
                                         wv_s[k][:],
                                         start=(k == 0 and b == 0),
                                         stop=(k == KB - 1))
                    if k == KB - 1:
                        nc.vector.tensor_copy(v_s[14][:, :, 0:DH],
                                              pv2[:, 0, :, :])
                        nc.vector.tensor_copy(v_s[15][:, :, 0:DH],
                                              pv2[:, 1, :, :])
                vunits.append(vunit)

            # rope(2)/(3) fillers pop during q0 (k-streams first: krot gates
            # attention); their shift tiles ride the sc pool's slots
            pending = [lambda nn=nn, ri=ri: emit_rope(nn, only=(ri,), ps=sc_ps,
                                                      ps_tag="sc")
                       for ri in (0, 3, 1, 2) for nn in (NS - 2, NS - 1)]
            finish_prev = [None]  # deferred transpose+evict of previous (h,q)

            def emit_quarter(mb, qt, eng):
                msl = slice(128 * mb, 128 * (mb + 1))
                cols = slice(512 * qt, 512 * (qt + 1))
                yp = yp_ps.tile([128, 512], F32, name=f"yp{mb}_{qt}", tag="yp")
                nc.tensor.matmul(yp[:], oT_s[0][:, msl], wo_s[0][:, cols],
                                 start=True, stop=False)
                nc.tensor.matmul(yp[:], oT_s[1][:, msl], wo_s[1][:, cols],
                                 start=False, stop=True)
                ys = ysb_pool.tile([128, 512], BF16, name=f"ys{mb}_{qt}", tag="ys")
                nc.vector.tensor_copy(ys[:], yp[:])
                if eng == 0:
                    nc.sync.dma_start(out=y_d[mb][:, cols], in_=ys[:])
                else:
                    nc.gpsimd.dma_start(out=y_d[mb][:, cols], in_=ys[:])

            for q in range(NS):
                qs = slice(512 * q, 512 * (q + 1))
                for h in range(HPC):
                    ot = [ops_ps.tile([128, 2, DH + 1], F32,
                                      name=f"osum{q}_{h}_{t}", tag="ops")
                          for t in range(2)]
                    pv_queue = []  # PV runs 2 groups behind sc to hide exp latency
                    for g in range(8):
                        sc_t = sc_ps.tile([128, 1024], F32,
                                          name=f"sc{q}_{h}_{g}", tag="sc")
                        nc.tensor.matmul(sc_t[:, 0:512], krot[h][:, ksl(2 * g)],
                                         qrot[h][:, qs], start=True, stop=True)
                        nc.tensor.matmul(sc_t[:, 512:1024],
                                         krot[h][:, ksl(2 * g + 1)],
                                         qrot[h][:, qs], start=True, stop=True)
                        pr_t = pr_pool.tile([128, 1024], BF16,
                                            name=f"pr{q}_{h}_{g}", tag="pr")
                        nc.scalar.activation(pr_t[:], sc_t[:], AF.Exp,
                                             scale=SCALE)
                        chunk = (lambda half, c, pr_t=pr_t:
                                 pr_t[:, 512 * half + 128 * c:
                                      512 * half + 128 * (c + 1)])
                        if DEBUG and q == 0 and h == 0 and g == 0:
                            nc.sync.dma_start(out=dbg_pr[:], in_=pr_t[:])
                        if g == 3 and finish_prev[0] is not None:
                            finish_prev[0]()
                            finish_prev[0] = None
                        # interleave fillers: at q0, rope(2)/(3) units every
                        # group plus 2 deferred-v units per h0 group; at q1+,
                        # yp quarters (skip g7 to keep DVE clear for the
                        # h-boundary normalize chain)
                        def may_pop():
                            return pending and not (q == NS - 1
                                                    and len(pending) <= 3)
                        if q == 0:
                            if pending:
                                pending.pop(0)()
                            if h == 0 and vunits:
                                vunits.pop(0)()
                                if vunits:
                                    vunits.pop(0)()
                        elif not (h == 0 and g <= 3) and g != 7:
                            if may_pop():
                                pending.pop(0)()
                            if g in (2, 4, 5) and may_pop():
                                pending.pop(0)()

                        def pv_emit(g=g, chunk=chunk):
                            for half in range(2):
                                kb = 2 * g + half
                                for c in range(4):
                                    nc.tensor.matmul(
                                        ot[c // 2][:, c % 2, :],
                                        chunk(half, c), v_s[kb][:, h, :],
                                        start=(kb == 0 and c % 2 == 0),
                                        stop=(kb == KB - 1))
                        pv_queue.append(pv_emit)
                        if len(pv_queue) > 2:
                            pv_queue.pop(0)()
                    for f in pv_queue:
                        f()

                    # normalization: rinv from the fused row-sum column, applied
                    # per-partition while evicting to bf16
                    rinv_t = [rinv_pool.tile([128, 2, 1], F32,
                                             name=f"rinv{q}_{h}_{t}", tag="rinv")
                              for t in range(2)]
                    for t in range(2):
                        nc.vector.reciprocal_approx_fast(rinv_t[t][:, :, :],
                                                         ot[t][:, :, DH:DH + 1])
                    onat = onat_pool.tile([128, 4, DH], BF16, name=f"onat{q}_{h}",
                                          tag="onat")
                    for c in range(4):
                        nc.vector.tensor_scalar_mul(onat[:, c, :],
                                                    ot[c // 2][:, c % 2, 0:DH],
                                                    rinv_t[c // 2][:, c % 2, :])
                    if DEBUG and q == 0 and h == 0:
                        for t in range(2):
                            nc.sync.dma_start(out=dbg_rinv[t],
                                              in_=rinv_t[t][:, :, 0])
                        nc.sync.dma_start(out=dbg_onat[:], in_=onat[:])
                    tr = sc_ps.tile([128, 4, DH], BF16, name=f"tr{q}_{h}", tag="sc")

                    def finish(q=q, h=h, onat=onat, tr=tr):
                        for c in range(4):
                            nc.tensor.matmul(tr[:, c, :], onat[:, c, :],
                                             ident_s[:], is_transpose=True,
                                             start=(c == 0), stop=(c == 3))
                        for c in range(4):
                            cols = slice(512 * q + 128 * c, 512 * q + 128 * (c + 1))
                            nc.vector.tensor_copy(oT_s[h][:, cols], tr[:, c, :])
                    finish_prev[0] = finish

                # queue this q-slice's output-projection quarters (the last
                # q-slice is instead emitted as double-width halves in the tail)
                if q < NS - 1:
                    eng = 0
                    for mb in range(4 * q, 4 * q + 4):
                        for qt in range(4):
                            pending.append(lambda mb=mb, qt=qt, e=eng:
                                           emit_quarter(mb, qt, e))
                            eng ^= 1

            # tail: the 3 reserved q2 quarters cover the PE gap while the last
            # (h,q)'s normalize chain completes, then its transposes run, then
            # the last q-slice's output projection streams as [128,1024] halves
            # through the freed sc-pool slots (double-buffered, no evict stall).
            for p in pending:
                p()
            pending = []
            finish_prev[0]()
            finish_prev[0] = None
            if DEBUG:
                for hh in range(HPC):
                    nc.sync.dma_start(out=dbg_oT[hh], in_=oT_s[hh][:])
            for mb in range(4 * (NS - 1), 4 * NS):
                for hf in range(2):
                    yph = sc_ps.tile([128, 1024], F32, name=f"yph{mb}_{hf}", tag="sc")
                    for h in range(HPC):
                        for nn in range(2):
                            cols = slice(1024 * hf + 512 * nn,
                                         1024 * hf + 512 * (nn + 1))
                            nc.tensor.matmul(yph[:, 512 * nn:512 * (nn + 1)],
                                             oT_s[h][:, slice(128 * mb, 128 * (mb + 1))],
                                             wo_s[h][:, cols],
                                             start=(h == 0), stop=(h == HPC - 1))
                    ysh = ysb_pool.tile([128, 1024], BF16, name=f"ysh{mb}_{hf}",
                                        tag="ysh")
                    if (2 * mb + hf) % 2 == 0:
                        nc.scalar.copy(ysh[:], yph[:])
                    else:
                        nc.vector.tensor_copy(ysh[:], yph[:])
                    nc.sync.dma_start(out=y_d[mb][:, 1024 * hf:1024 * hf + 512],
                                      in_=ysh[:, 0:512])
                    nc.gpsimd.dma_start(out=y_d[mb][:, 1024 * hf + 512:1024 * (hf + 1)],
                                        in_=ysh[:, 512:1024])
            ph3.close()

    nc.compile()
    return nc


def _prepare_inputs(hidden_states, wq, wk, wv, wo, position_ids, s, d):
    """Host-side sharding/layout prep. Returns per-core input maps."""
    import ml_dtypes

    x = np.asarray(hidden_states, np.float32).reshape(s, d)
    kb = d // 128
    # partition-major bf16: [128 rows-within-chunk, kb, s]
    xT = np.ascontiguousarray(
        x.T.reshape(kb, 128, s).transpose(1, 0, 2)).astype(ml_dtypes.bfloat16)

    pos = np.asarray(position_ids).reshape(-1)[:s].astype(np.float64)
    inv_freq = 1.0 / (ROPE_BASE ** (np.arange(0, DH, 2, dtype=np.float64) / DH))
    freqs = np.outer(pos, inv_freq)                      # [s, dh/2]
    emb = np.concatenate([freqs, freqs], axis=-1)        # [s, dh]
    cosT = np.ascontiguousarray(np.cos(emb).T.astype(np.float32))   # [dh, s]
    sinT = np.ascontiguousarray(np.sin(emb).T.astype(np.float32))
    ssinT = sinT.copy()
    ssinT[: DH // 2] *= -1.0

    perm64 = np.zeros((128, 128), np.float32)
    for m in range(128):
        perm64[(m + 64) % 128, m] = 1.0
    ident = np.eye(128, dtype=ml_dtypes.bfloat16)

    wq = np.asarray(wq, np.float32)
    wk = np.asarray(wk, np.float32)
    wv = np.asarray(wv, np.float32)
    wo = np.asarray(wo, np.float32)

    in_maps = []
    for c in range(NCORES):
        cs = slice(DHC * c, DHC * (c + 1))
        wqT = wq[cs, :].T.reshape(kb, 128, DHC)
        wkT = wk[cs, :].T.reshape(kb, 128, DHC)
        wvT = wv[cs, :].T.reshape(kb, 128, DHC)
        # packed bf16 [128, kb, 3, DHC]
        wqkvT = np.ascontiguousarray(
            np.stack([wqT, wkT, wvT], axis=1).transpose(2, 0, 1, 3)
        ).astype(ml_dtypes.bfloat16)
        woT = np.ascontiguousarray(wo[:, cs].T).reshape(HPC, 128, d)
        woT = woT.astype(ml_dtypes.bfloat16)
        in_maps.append(dict(
            xT2=xT, wqkvT=wqkvT, woT=woT,
            cosT=cosT, ssinT=ssinT,
            perm64=perm64, ident=ident,
        ))
    return in_maps


def kernel(hidden_states, wq, wk, wv, wo, position_ids):
    from concourse.bass_utils import run_bass_kernel_spmd

    b, s, d = hidden_states.shape
    if "nc" not in _CACHE:
        _CACHE["nc"] = _build(s, d)
    nc = _CACHE["nc"]

    in_maps = _prepare_inputs(hidden_states, wq, wk, wv, wo, position_ids, s, d)
    res = None
    last_err = None
    for attempt in range(3):
        try:
            res = run_bass_kernel_spmd(nc, in_maps, core_ids=list(range(NCORES)))
            break
        except Exception as e:  # transient device/terminal failures happen
            last_err = e
            import time as _time
            _time.sleep(5.0)
    if res is None:
        raise last_err
    y = np.zeros((s, d), np.float64)
    for c in range(NCORES):
        y += res.results[c]["y"].reshape(s, d).astype(np.float64)
    return y.astype(np.float32).reshape(b, s, d)


# revision 88
# speedup vs baseline: 1.0273x; 1.0229x over previous
"""LLaMA attention block (b=1, s=2048, d=2048, 16 heads) on 8 TRN2 NeuronCores.

Sharding: tensor-parallel over heads (2 heads per core). Each core computes
q/k/v projections for its head slice, RoPE, full (non-causal) attention for its
heads, and a partial output projection; the host sums the 8 partial outputs.

Device-side layout notes (per core):
  - x is passed transposed (xT, d-major) so projections contract over the
    partition dim without on-device transposes.
  - q/k are produced transposed per head: qT/kT [dh=128, s], RoPE'd in place
    (interleaved with the projection loop so PE never idles).
  - scores are computed transposed: scoresT [k, q]; exp (Act engine) evicts
    PSUM->SBUF as bf16 probs.
  - PV + row-sum are FUSED: rhs = [v | ones] (bf16 [128,129]); lhsT = probsT
    128-q chunk. Output osum is [q, dh+1] with the row-sum in column dh.
    This replaces the old M=1 ones-matmul (512 rows/block -> 1 row/block).
  - normalization is a per-partition scalar multiply at PSUM eviction
    (DVE tensor_scalar with rinv[128,1]); a cheap PE transpose (bf16,
    128 rows) restores the [dh, s] layout for the output projection.
  - probs/v/attn-out/wo are bf16 (rel err ~3e-3 vs gate 2e-2); projections
    and scores stay fp32r (full PE rate at N>=256).
  - output projection is interleaved into the attention loop (one [128,512]
    yp quarter per score group) so the PE stream has no phase boundary gaps
    (PE p-state halves the clock for 3us after any idle gap).
"""
import numpy as np
from contextlib import ExitStack

S, D, NH, DH = 2048, 2048, 16, 128
NCORES = 8
HPC = NH // NCORES          # heads per core
DHC = HPC * DH              # per-core projection width (256)
ROPE_BASE = 10000.0

_CACHE = {}
DEBUG = False


def _build(s, d):
    import concourse.bacc as bacc
    import concourse.mybir as mybir
    import concourse.tile as tile

    F32 = mybir.dt.float32
    F32R = mybir.dt.float32r
    BF16 = mybir.dt.bfloat16
    AF = mybir.ActivationFunctionType

    KB = d // 128          # contraction chunks for projections
    NS = s // 512          # s-slices for projections / q-slices for attention
    MB = s // 128          # s-blocks for output rows
    SCALE = 1.0 / float(np.sqrt(DH))

    def ksl(kb):
        return slice(128 * kb, 128 * (kb + 1))

    nc = bacc.Bacc("TRN2", target_bir_lowering=False, debug=False)

    # x and the packed qkv weights are partition-major so multi-chunk DMAs
    # iterate [p, chunk, ...] on both sides (fewer DMAs -> less per-DMA
    # fixed overhead), and bf16 (DMA transfers serialize globally in HW;
    # phase 1 is DMA-limited at fp32). bf16 projections still run 1 cyc/row.
    xT_d = nc.dram_tensor("xT2", [128, KB, s], BF16, kind="ExternalInput")
    wqkv_d = nc.dram_tensor("wqkvT", [128, KB, 3, DHC], BF16, kind="ExternalInput")
    wo_d = nc.dram_tensor("woT", [HPC, 128, s], BF16, kind="ExternalInput")
    cos_d = nc.dram_tensor("cosT", [128, s], F32, kind="ExternalInput")
    ssin_d = nc.dram_tensor("ssinT", [128, s], F32, kind="ExternalInput")
    perm_d = nc.dram_tensor("perm64", [128, 128], F32, kind="ExternalInput")
    ident_d = nc.dram_tensor("ident", [128, 128], BF16, kind="ExternalInput")
    # y partials in bf16: DMA transfers serialize globally, and the final
    # output-projection tail is DMA-bound at fp32 (host sums partials in f64)
    y_d = nc.dram_tensor("y", [MB, 128, s], BF16, kind="ExternalOutput")
    if DEBUG:
        dbg_rot = nc.dram_tensor("dbg_rot", [4, 128, s], F32, kind="ExternalOutput")
        dbg_v = nc.dram_tensor("dbg_v", [MB, 128, HPC, DH + 1], BF16,
                               kind="ExternalOutput")
        dbg_pr = nc.dram_tensor("dbg_pr", [128, 1024], BF16, kind="ExternalOutput")
        dbg_onat = nc.dram_tensor("dbg_onat", [128, 4, DH], BF16,
                                  kind="ExternalOutput")
        dbg_rinv = nc.dram_tensor("dbg_rinv", [2, 128, 2], F32,
                                  kind="ExternalOutput")
        dbg_oT = nc.dram_tensor("dbg_oT", [HPC, 128, s], BF16,
                                kind="ExternalOutput")

    with tile.TileContext(nc) as tc:
        with ExitStack() as root:
            consts = root.enter_context(tc.tile_pool(name="consts", bufs=1))
            perm_s = consts.tile([128, 128], F32R, name="perm_s")
            ident_s = consts.tile([128, 128], BF16, name="ident_s")
            cos_s = consts.tile([128, s], F32, name="cos_s")
            ssin_s = consts.tile([128, s], F32, name="ssin_s")
            wo_pool = root.enter_context(tc.tile_pool(name="wo_pool", bufs=1))
            wo_s = [wo_pool.tile([128, s], BF16, name=f"wo{h}") for h in range(HPC)]

            # v tiles hold [v_head0 | 1 | v_head1 | 1]: the ones column turns
            # the PV matmul into a fused PV+rowsum (out free dim 129).
            v_pool = root.enter_context(tc.tile_pool(name="v_pool", bufs=1))
            v_s = [v_pool.tile([128, HPC, DH + 1], BF16, name=f"v{i}")
                   for i in range(MB)]
            for i in range(MB):
                nc.vector.memset(v_s[i][:, :, DH:DH + 1], 1.0)

            rot_pool = root.enter_context(tc.tile_pool(name="rot_pool", bufs=1))
            qrot = [rot_pool.tile([128, s], F32R, name=f"qrot{m}") for m in range(HPC)]
            krot = [rot_pool.tile([128, s], F32R, name=f"krot{m}") for m in range(HPC)]

            oT_pool = root.enter_context(tc.tile_pool(name="oT_pool", bufs=1))
            oT_s = [oT_pool.tile([128, s], BF16, name=f"oT{h}") for h in range(HPC)]

            # qT/kT and the rope scratch pools live at root scope: the q1/k1
            # rope of the last two slices is deferred into early attention
            qkpre = root.enter_context(tc.tile_pool(name="qkpre", bufs=1))
            qT_s = [qkpre.tile([128, s], F32R, name=f"qT{m}") for m in range(HPC)]
            kT_s = [qkpre.tile([128, s], F32R, name=f"kT{m}") for m in range(HPC)]
            t1_pool = root.enter_context(tc.tile_pool(name="t1_pool", bufs=2))
            t2_pool = root.enter_context(tc.tile_pool(name="t2_pool", bufs=2))

            wqkv = root.enter_context(tc.tile_pool(name="wqkv", bufs=1))
            wqkv_s = wqkv.tile([128, KB, 3, DHC], BF16, name="wqkv_s")

            # ---------- phase 1: q/k/v projections with RoPE interleaved ----------
            ph1 = ExitStack()
            # kb 0 alone first so the PE can start as early as possible
            nc.scalar.dma_start(out=wqkv_s[:, 0:1, :, :], in_=wqkv_d[:, 0:1, :, :])
            nc.scalar.dma_start(out=wqkv_s[:, 1:2, :, :], in_=wqkv_d[:, 1:2, :, :])
            for p in range(1, KB // 2):
                nc.scalar.dma_start(out=wqkv_s[:, 2 * p:2 * p + 2, :, :],
                                    in_=wqkv_d[:, 2 * p:2 * p + 2, :, :])
            wq_s = [wqkv_s[:, i, 0, :] for i in range(KB)]
            wk_s = [wqkv_s[:, i, 1, :] for i in range(KB)]
            wv_s = [wqkv_s[:, i, 2, :] for i in range(KB)]

            xk_pool = ph1.enter_context(tc.tile_pool(name="xk_pool", bufs=3))
            qk_ps = ph1.enter_context(tc.tile_pool(name="qk_ps", bufs=1, space="PSUM"))
            v_ps = ph1.enter_context(tc.tile_pool(name="v_ps", bufs=1, space="PSUM"))
            rope_ps = ph1.enter_context(tc.tile_pool(name="rope_ps", bufs=2, space="PSUM"))

            # rope streams: k-head0 first so attention can start earliest
            streams = [(kT_s[0], krot[0]), (qT_s[0], qrot[0]),
                       (qT_s[1], qrot[1]), (kT_s[1], krot[1])]

            def emit_rope(nidx, only=(0, 1, 2, 3), ps=None, ps_tag="shift"):
                ns_ = slice(512 * nidx, 512 * (nidx + 1))
                for ri, (src, dst) in enumerate(streams):
                    if ri not in only:
                        continue
                    shift = (ps or rope_ps).tile([128, 512], F32,
                                                 name=f"sh{ri}_{nidx}", tag=ps_tag)
                    nc.tensor.matmul(shift[:], perm_s[:], src[:, ns_],
                                     start=True, stop=True)
                    t1 = t1_pool.tile([128, 512], F32, name=f"t1_{ri}_{nidx}", tag="t1")
                    nc.gpsimd.tensor_mul(t1[:], src[:, ns_].bitcast(F32), cos_s[:, ns_])
                    t2 = t2_pool.tile([128, 512], F32, name=f"t2_{ri}_{nidx}", tag="t2")
                    nc.vector.tensor_mul(t2[:], shift[:], ssin_s[:, ns_])
                    nc.vector.tensor_add(dst[:, ns_], t1[:], t2[:])

            for n in range(NS):
                ns_ = slice(512 * n, 512 * (n + 1))
                pq = [qk_ps.tile([128, 512], F32, name=f"pq{n}_{m}", tag=f"pq{m}")
                      for m in range(HPC)]
                pk = [qk_ps.tile([128, 512], F32, name=f"pk{n}_{m}", tag=f"pk{m}")
                      for m in range(HPC)]
                pv = [v_ps.tile([128, 2, 2, 128], F32, name=f"pv{n}_{t}", tag=f"pv{t}")
                      for t in range(2)]
                xkq = None
                for kb in range(KB):
                    if kb % 4 == 0:
                        xkq = xk_pool.tile([128, 4, 512], BF16,
                                           name=f"xk{n}_{kb // 4}", tag="xk")
                        if n == 0 and kb == 0:
                            # split so kb 0 lands quickly at startup
                            nc.sync.dma_start(out=xkq[:, 0:1, :],
                                              in_=xT_d[:, 0:1, ns_])
                            nc.sync.dma_start(out=xkq[:, 1:4, :],
                                              in_=xT_d[:, 1:4, ns_])
                        else:
                            nc.sync.dma_start(out=xkq[:],
                                              in_=xT_d[:, kb:kb + 4, ns_])
                    if kb == 8:
                        # cos/ssin arrive per-slice, interleaved between the
                        # x quads (slice n's columns are first read by rope(n)
                        # at the end of slice n+1's projection loop)
                        nc.scalar.dma_start(out=cos_s[:, ns_], in_=cos_d[:, ns_])
                        nc.sync.dma_start(out=ssin_s[:, ns_], in_=ssin_d[:, ns_])
                        if n == 0:
                            nc.scalar.dma_start(out=perm_s[:],
                                                in_=perm_d[:].bitcast(F32R))
                    xk = xkq[:, kb % 4, :]
                    st = kb == 0
                    sp = kb == KB - 1

                    def mm_q(m):
                        ms = slice(128 * m, 128 * (m + 1))
                        nc.tensor.matmul(pq[m][:], wq_s[kb][:, ms], xk,
                                         start=st, stop=sp)

                    def mm_k(m):
                        ms = slice(128 * m, 128 * (m + 1))
                        nc.tensor.matmul(pk[m][:], wk_s[kb][:, ms], xk,
                                         start=st, stop=sp)

                    def mm_v(j):
                        # psum start zeroes the whole bank: only the bank's
                        # first write (slot 0, kb 0) may set start=True
                        js = slice(128 * j, 128 * (j + 1))
                        nc.tensor.matmul(pv[j // 2][:, j % 2, :, :], xk[:, js],
                                         wv_s[kb][:], start=(st and j % 2 == 0),
                                         stop=sp)

                    if kb == 0 and n > 0:
                        # order matched to eviction completion of slice n-1
                        # (Act: pq0,pq1,pk1 / DVE: pk0 then pv0..3)
                        for f in (lambda: mm_q(0), lambda: mm_k(0),
                                  lambda: mm_q(1), lambda: mm_k(1),
                                  lambda: mm_v(0), lambda: mm_v(1),
                                  lambda: mm_v(2), lambda: mm_v(3)):
                            f()
                    else:
                        mm_q(0), mm_q(1), mm_k(0), mm_k(1)
                        for j in range(4):
                            mm_v(j)

                # evictions (only Act/DVE may read PSUM):
                # Act: pq0, pq1, pk1; DVE: pk0 first (rope k0 gate), then v.
                nc.vector.tensor_copy(kT_s[0][:, ns_], pk[0][:])
                nc.scalar.copy(qT_s[0][:, ns_], pq[0][:])
                nc.vector.tensor_copy(kT_s[1][:, ns_], pk[1][:])
                nc.scalar.copy(qT_s[1][:, ns_], pq[1][:])
                for j in range(4):
                    nc.vector.tensor_copy(v_s[4 * n + j][:, :, 0:DH],
                                          pv[j // 2][:, j % 2, :, :])
                if n > 0:
                    emit_rope(n - 1, only=(0, 1, 2, 3) if n < NS - 1 else (0, 1))
                if n == NS - 1:
                    emit_rope(n, only=(0, 1))
            nc.sync.dma_start(out=ident_s[:], in_=ident_d[:])
            for h in range(HPC):
                nc.scalar.dma_start(out=wo_s[h][:], in_=wo_d[h])
            ph1.close()

            # -------- phase 3+4: attention with output projection interleaved ----
            ph3 = ExitStack()
            sc_ps = ph3.enter_context(tc.tile_pool(name="sc_ps", bufs=2, space="PSUM"))
            ops_ps = ph3.enter_context(tc.tile_pool(name="ops_ps", bufs=2, space="PSUM"))
            yp_ps = ph3.enter_context(tc.tile_pool(name="yp_ps", bufs=2, space="PSUM"))
            pr_pool = ph3.enter_context(tc.tile_pool(name="pr_pool", bufs=3))
            onat_pool = ph3.enter_context(tc.tile_pool(name="onat_pool", bufs=2))
            rinv_pool = ph3.enter_context(tc.tile_pool(name="rinv_pool", bufs=4))
            ysb_pool = ph3.enter_context(tc.tile_pool(name="ysb_pool", bufs=8))

            if DEBUG:
                for ri, (_, dst) in enumerate(streams):
                    nc.sync.dma_start(out=dbg_rot[ri], in_=dst[:].bitcast(F32))
                for i in range(MB):
                    nc.sync.dma_start(out=dbg_v[i], in_=v_s[i][:])

            # pending: filler emitters the PE can chew at group boundaries —
            # first the deferred q1/k1 rope, later prev-q-slice yp quarters
            pending = [lambda nn=nn, ri=ri: emit_rope(nn, only=(ri,), ps=yp_ps,
                                                      ps_tag="yp")
                       for nn in (NS - 2, NS - 1) for ri in (2, 3)]
            finish_prev = [None]  # deferred transpose+evict of previous (h,q)

            def emit_quarter(mb, qt, eng):
                msl = slice(128 * mb, 128 * (mb + 1))
                cols = slice(512 * qt, 512 * (qt + 1))
                yp = yp_ps.tile([128, 512], F32, name=f"yp{mb}_{qt}", tag="yp")
                nc.tensor.matmul(yp[:], oT_s[0][:, msl], wo_s[0][:, cols],
                                 start=True, stop=False)
                nc.tensor.matmul(yp[:], oT_s[1][:, msl], wo_s[1][:, cols],
                                 start=False, stop=True)
                ys = ysb_pool.tile([128, 512], BF16, name=f"ys{mb}_{qt}", tag="ys")
                nc.vector.tensor_copy(ys[:], yp[:])
                if eng == 0:
                    nc.sync.dma_start(out=y_d[mb][:, cols], in_=ys[:])
                else:
                    nc.gpsimd.dma_start(out=y_d[mb][:, cols], in_=ys[:])

            for q in range(NS):
                qs = slice(512 * q, 512 * (q + 1))
                for h in range(HPC):
                    ot = [ops_ps.tile([128, 2, DH + 1], F32,
                                      name=f"osum{q}_{h}_{t}", tag="ops")
                          for t in range(2)]
                    pv_queue = []  # PV runs 2 groups behind sc to hide exp latency
                    for g in range(8):
                        sc_t = sc_ps.tile([128, 1024], F32,
                                          name=f"sc{q}_{h}_{g}", tag="sc")
                        nc.tensor.matmul(sc_t[:, 0:512], krot[h][:, ksl(2 * g)],
                                         qrot[h][:, qs], start=True, stop=True)
                        nc.tensor.matmul(sc_t[:, 512:1024],
                                         krot[h][:, ksl(2 * g + 1)],
                                         qrot[h][:, qs], start=True, stop=True)
                        pr_t = pr_pool.tile([128, 1024], BF16,
                                            name=f"pr{q}_{h}_{g}", tag="pr")
                        nc.scalar.activation(pr_t[:], sc_t[:], AF.Exp,
                                             scale=SCALE)
                        chunk = (lambda half, c, pr_t=pr_t:
                                 pr_t[:, 512 * half + 128 * c:
                                      512 * half + 128 * (c + 1)])
                        if DEBUG and q == 0 and h == 0 and g == 0:
                            nc.sync.dma_start(out=dbg_pr[:], in_=pr_t[:])
                        if g == 3 and finish_prev[0] is not None:
                            finish_prev[0]()
                            finish_prev[0] = None
                        # interleave fillers (rope tail / yp quarters); skip
                        # g7 so the DVE queue is clear for the h-boundary
                        # normalize chain; at q0 only during h1 (rope fillers'
                        # psum bank is held by ph1 rope reads during h0)
                        def may_pop():
                            return pending and not (q == NS - 1
                                                    and len(pending) <= 3)
                        if not (h == 0 and (g <= 3 or q == 0)) and g != 7:
                            if may_pop():
                                pending.pop(0)()
                            if g in (2, 4, 5) and may_pop():
                                pending.pop(0)()

                        def pv_emit(g=g, chunk=chunk):
                            for half in range(2):
                                kb = 2 * g + half
                                for c in range(4):
                                    nc.tensor.matmul(
                                        ot[c // 2][:, c % 2, :],
                                        chunk(half, c), v_s[kb][:, h, :],
                                        start=(kb == 0 and c % 2 == 0),
                                        stop=(kb == KB - 1))
                        pv_queue.append(pv_emit)
                        if len(pv_queue) > 2:
                            pv_queue.pop(0)()
                    for f in pv_queue:
                        f()

                    # normalization: rinv from the fused row-sum column, applied
                    # per-partition while evicting to bf16
                    rinv_t = [rinv_pool.tile([128, 2, 1], F32,
                                             name=f"rinv{q}_{h}_{t}", tag="rinv")
                              for t in range(2)]
                    for t in range(2):
                        nc.vector.reciprocal_approx_fast(rinv_t[t][:, :, :],
                                                         ot[t][:, :, DH:DH + 1])
                    onat = onat_pool.tile([128, 4, DH], BF16, name=f"onat{q}_{h}",
                                          tag="onat")
                    for c in range(4):
                        nc.vector.tensor_scalar_mul(onat[:, c, :],
                                                    ot[c // 2][:, c % 2, 0:DH],
                                                    rinv_t[c // 2][:, c % 2, :])
                    if DEBUG and q == 0 and h == 0:
                        for t in range(2):
                            nc.sync.dma_start(out=dbg_rinv[t],
                                              in_=rinv_t[t][:, :, 0])
                        nc.sync.dma_start(out=dbg_onat[:], in_=onat[:])
                    tr = sc_ps.tile([128, 4, DH], BF16, name=f"tr{q}_{h}", tag="sc")

                    def finish(q=q, h=h, onat=onat, tr=tr):
                        for c in range(4):
                            nc.tensor.matmul(tr[:, c, :], onat[:, c, :],
                                             ident_s[:], is_transpose=True,
                                             start=(c == 0), stop=(c == 3))
                        for c in range(4):
                            cols = slice(512 * q + 128 * c, 512 * q + 128 * (c + 1))
                            nc.vector.tensor_copy(oT_s[h][:, cols], tr[:, c, :])
                    finish_prev[0] = finish

                # queue this q-slice's output-projection quarters (the last
                # q-slice is instead emitted as double-width halves in the tail)
                if q < NS - 1:
                    eng = 0
                    for mb in range(4 * q, 4 * q + 4):
                        for qt in range(4):
                            pending.append(lambda mb=mb, qt=qt, e=eng:
                                           emit_quarter(mb, qt, e))
                            eng ^= 1

            # tail: the 3 reserved q2 quarters cover the PE gap while the last
            # (h,q)'s normalize chain completes, then its transposes run, then
            # the last q-slice's output projection streams as [128,1024] halves
            # through the freed sc-pool slots (double-buffered, no evict stall).
            for p in pending:
                p()
            pending = []
            finish_prev[0]()
            finish_prev[0] = None
            if DEBUG:
                for hh in range(HPC):
                    nc.sync.dma_start(out=dbg_oT[hh], in_=oT_s[hh][:])
            for mb in range(4 * (NS - 1), 4 * NS):
                for hf in range(2):
                    yph = sc_ps.tile([128, 1024], F32, name=f"yph{mb}_{hf}", tag="sc")
                    for h in range(HPC):
                        for nn in range(2):
                            cols = slice(1024 * hf + 512 * nn,
                                         1024 * hf + 512 * (nn + 1))
                            nc.tensor.matmul(yph[:, 512 * nn:512 * (nn + 1)],
                                             oT_s[h][:, slice(128 * mb, 128 * (mb + 1))],
                                             wo_s[h][:, cols],
                                             start=(h == 0), stop=(h == HPC - 1))
                    ysh = ysb_pool.tile([128, 1024], BF16, name=f"ysh{mb}_{hf}",
                                        tag="ysh")
                    if (2 * mb + hf) % 2 == 0:
                        nc.scalar.copy(ysh[:], yph[:])
                    else:
                        nc.vector.tensor_copy(ysh[:], yph[:])
                    nc.sync.dma_start(out=y_d[mb][:, 1024 * hf:1024 * hf + 512],
                                      in_=ysh[:, 0:512])
                    nc.gpsimd.dma_start(out=y_d[mb][:, 1024 * hf + 512:1024 * (hf + 1)],
                                        in_=ysh[:, 512:1024])
            ph3.close()

    nc.compile()
    return nc


def _prepare_inputs(hidden_states, wq, wk, wv, wo, position_ids, s, d):
    """Host-side sharding/layout prep. Returns per-core input maps."""
    import ml_dtypes

    x = np.asarray(hidden_states, np.float32).reshape(s, d)
    kb = d // 128
    # partition-major bf16: [128 rows-within-chunk, kb, s]
    xT = np.ascontiguousarray(
        x.T.reshape(kb, 128, s).transpose(1, 0, 2)).astype(ml_dtypes.bfloat16)

    pos = np.asarray(position_ids).reshape(-1)[:s].astype(np.float64)
    inv_freq = 1.0 / (ROPE_BASE ** (np.arange(0, DH, 2, dtype=np.float64) / DH))
    freqs = np.outer(pos, inv_freq)                      # [s, dh/2]
    emb = np.concatenate([freqs, freqs], axis=-1)        # [s, dh]
    cosT = np.ascontiguousarray(np.cos(emb).T.astype(np.float32))   # [dh, s]
    sinT = np.ascontiguousarray(np.sin(emb).T.astype(np.float32))
    ssinT = sinT.copy()
    ssinT[: DH // 2] *= -1.0

    perm64 = np.zeros((128, 128), np.float32)
    for m in range(128):
        perm64[(m + 64) % 128, m] = 1.0
    ident = np.eye(128, dtype=ml_dtypes.bfloat16)

    wq = np.asarray(wq, np.float32)
    wk = np.asarray(wk, np.float32)
    wv = np.asarray(wv, np.float32)
    wo = np.asarray(wo, np.float32)

    in_maps = []
    for c in range(NCORES):
        cs = slice(DHC * c, DHC * (c + 1))
        wqT = wq[cs, :].T.reshape(kb, 128, DHC)
        wkT = wk[cs, :].T.reshape(kb, 128, DHC)
        wvT = wv[cs, :].T.reshape(kb, 128, DHC)
        # packed bf16 [128, kb, 3, DHC]
        wqkvT = np.ascontiguousarray(
            np.stack([wqT, wkT, wvT], axis=1).transpose(2, 0, 1, 3)
        ).astype(ml_dtypes.bfloat16)
        woT = np.ascontiguousarray(wo[:, cs].T).reshape(HPC, 128, d)
        woT = woT.astype(ml_dtypes.bfloat16)
        in_maps.append(dict(
            xT2=xT, wqkvT=wqkvT, woT=woT,
            cosT=cosT, ssinT=ssinT,
            perm64=perm64, ident=ident,
        ))
    return in_maps


def kernel(hidden_states, wq, wk, wv, wo, position_ids):
    from concourse.bass_utils import run_bass_kernel_spmd

    b, s, d = hidden_states.shape
    if "nc" not in _CACHE:
        _CACHE["nc"] = _build(s, d)
    nc = _CACHE["nc"]

    in_maps = _prepare_inputs(hidden_states, wq, wk, wv, wo, position_ids, s, d)
    res = None
    last_err = None
    for attempt in range(3):
        try:
            res = run_bass_kernel_spmd(nc, in_maps, core_ids=list(range(NCORES)))
            break
        except Exception as e:  # transient device/terminal failures happen
            last_err = e
            import time as _time
            _time.sleep(5.0)
    if res is None:
        raise last_err
    y = np.zeros((s, d), np.float64)
    for c in range(NCORES):
        y += res.results[c]["y"].reshape(s, d).astype(np.float64)
    return y.astype(np.float32).reshape(b, s, d)
